# revision 37
# baseline (speedup 1.0000x reference)
"""BrainQuantumLayer Trainium2 kernel.

Data-parallel over the 4096-token dimension across 8 NeuronCores
(512 tokens/core); the 2048x2048 recurrence matrices are replicated.

On-chip layout is feature-major ("transposed"): state lives as
stateT[n, tok] so both recurrence matmuls keep the weight matrices as
the PE-stationary operand and the token dimension as the moving free
dim (N=512, one PSUM bank). All matmul operands are fp16 (11-bit
mantissa; ~4e-3 scale-relative output error vs the fp32 reference,
measured) which runs the PE at full 1-cycle/row rate; PSUM accumulation
is fp32 and the epilogue arithmetic is fp32.

Per core, per time step (16 output-blocks ncb):
  psA = sum_k eff_w[k][:, ncb]   @ stateT[k]      (signal, 16 MMs)
  psB = sum_k J_lam_m[k -> ncb]  @ sT[k]          (delta,  16 MMs)
  sn  = noise*T01 + psA ; d = psB*s ; d += sn     (DVE)
  state'[ncb] = tanh(d) ; s'[ncb] = tanh(state'[ncb])   (ACT)

weights, J and mask all arrive host-permuted to the block-column
layout [ncb, p, k, c] that matches per-group consumption, so every
group depends on one contiguous ~0.5 MB DMA rather than a whole-matrix
prefix. eff_w = weights*mask is built inside step 0's groups (one mask
load serves both weight paths) and stays resident (8 MB fp16) for
steps 1-2; J*mask is rebuilt each step; lam is folded into the fp32
epilogue. State uses a 3-buffer rotation (state_t, s_t, next); s_t is
computed at step start into the dead buffer, B-groups trail A-groups
by one so the in-order PE never waits on the tanh chain, and a short
warm-up matmul block fills the initial DMA window while releasing the
PE clock gate.
"""

import numpy as np

TOKENS = 4096
N = 2048
IN_DIM = 1024
OUT_DIM = 1024
TIME_STEPS = 3
N_CORES = 8
TPC = TOKENS // N_CORES   # 512 tokens per core
P = 128
KC = N // P               # 16 n-chunks
KI = IN_DIM // P          # 8 input chunks
KO = OUT_DIM // P         # 8 output chunks

_PROG = None


def _build_program():
    import concourse.mybir as mybir
    from concourse import bacc
    from concourse.tile import TileContext

    f16 = mybir.dt.float16
    f32 = mybir.dt.float32
    Alu = mybir.AluOpType
    Act = mybir.ActivationFunctionType

    nc = bacc.Bacc(target_bir_lowering=False)

    xT = nc.dram_tensor("xT", [IN_DIM, TPC], f16, kind="ExternalInput")
    w_in_blk = nc.dram_tensor("w_in_blk", [KC, P, KI, P], f16, kind="ExternalInput")
    consts_t = nc.dram_tensor("consts_t", [P, 2 * KC + KO + 1], f32,
                              kind="ExternalInput")
    w_blkd = nc.dram_tensor("w_blkd", [KC, P, KC, P], f16, kind="ExternalInput")
    j_blk = nc.dram_tensor("j_blk", [KC, P, KC, P], f16, kind="ExternalInput")
    m_blk = nc.dram_tensor("m_blk", [KC, P, KC, P], f16, kind="ExternalInput")
    noiseT = nc.dram_tensor("noiseT", [TIME_STEPS, N, TPC], f16, kind="ExternalInput")
    w_out_blk = nc.dram_tensor("w_out_blk", [KO, P, KC, P], f16, kind="ExternalInput")
    yT = nc.dram_tensor("yT", [OUT_DIM, TPC], f32, kind="ExternalOutput")

    with TileContext(nc) as tc:
        with tc.tile_pool(name="const", bufs=1) as cpool, \
             tc.tile_pool(name="effw", bufs=1) as wpool, \
             tc.tile_pool(name="state", bufs=1) as spool, \
             tc.tile_pool(name="xt", bufs=1) as xpool, \
             tc.tile_pool(name="jset", bufs=4) as wstp, \
             tc.tile_pool(name="wset", bufs=3) as wsetp, \
             tc.tile_pool(name="blkst", bufs=3) as blkp, \
             tc.tile_pool(name="noise", bufs=6) as npool, \
             tc.tile_pool(name="epi", bufs=6) as epool, \
             tc.tile_pool(name="yout", bufs=2) as ypool, \
             tc.tile_pool(name="psum", bufs=8, space="PSUM") as pspool:

            # ---- PE warm-up: ~35 dependency-free matmuls on zeros ----
            warm = cpool.tile([P, P], f16, tag="warm")
            nc.vector.memset(warm, 0.0)
            wps = pspool.tile([P, TPC], f32, tag="ps", name="warmps")
            for _ in range(35):
                nc.tensor.matmul(wps[:, :P], warm, warm, start=True, stop=True)

            # ---- x chunks (two strided DMAs: first half lands sooner) ----
            x_all = xpool.tile([P, KI, TPC], f16, tag="xall")
            x_r = xT.rearrange("(ki p) t -> p ki t", p=P)
            nc.sync.dma_start(x_all[:, :KI // 2, :], x_r[:, :KI // 2, :])
            nc.sync.dma_start(x_all[:, KI // 2:, :], x_r[:, KI // 2:, :])
            xts = [x_all[:, ki, :] for ki in range(KI)]

            # ---- constants (single packed DMA) ----
            consts = cpool.tile([P, 2 * KC + KO + 1], f32, tag="consts")
            nc.sync.dma_start(consts, consts_t[:, :])
            bin_sb = consts[:, 0:KC]
            bout_sb = consts[:, KC:KC + KO]
            th_sb = consts[:, KC + KO:2 * KC + KO]
            lam_sb = consts[:, 2 * KC + KO:2 * KC + KO + 1]
            # T01 = 0.1 * |sin(2*theta)|
            t01 = cpool.tile([P, KC], f32, tag="t01")
            nc.scalar.activation(t01, th_sb, Act.Sin, scale=2.0)
            nc.scalar.activation(t01, t01, Act.Abs)
            nc.vector.tensor_scalar_mul(t01, t01, 0.1)

            # ---- state rotation buffers ----
            stA = [spool.tile([P, TPC], f16, tag=f"sA{k}", name=f"sA{k}")
                   for k in range(KC)]
            stB = [spool.tile([P, TPC], f16, tag=f"sB{k}", name=f"sB{k}")
                   for k in range(KC)]
            stC = [spool.tile([P, TPC], f16, tag=f"sC{k}", name=f"sC{k}")
                   for k in range(KC)]

            # ---- input projection: state0 = x @ W_in.T + b_in ----
            # (emitted first so its DMAs lead the queues; weight-matrix
            # streaming overlaps the projection matmuls)
            for ncb in range(KC):
                wi = blkp.tile([P, KI, P], f16, tag="wi")
                nc.sync.dma_start(wi, w_in_blk[ncb])
                ps = pspool.tile([P, TPC], f32, tag="ps")
                for ki in range(KI):
                    nc.tensor.matmul(ps, wi[:, ki, :], xts[ki],
                                     start=(ki == 0), stop=(ki == KI - 1))
                # state0 via DVE (keeps ACT on the Tanh table exclusively)
                nc.vector.tensor_scalar_add(stA[ncb], ps, bin_sb[:, ncb:ncb + 1])
                nc.scalar.activation(stB[ncb], ps, Act.Tanh,
                                     bias=bin_sb[:, ncb:ncb + 1])

            # eff_w column-blocks are built inside step 0 (below) and stay
            # resident for steps 1-2; mask blocks are shared with the J path
            effw_blk = [None] * KC

            # ---- recurrence ----
            # J_lam_m column-blocks are rebuilt from j_blk/m_blk every step
            # (16 MB/step streamed; cheaper than a scratch round-trip and it
            # keeps step-0 DMA pressure down)
            cur, curs, spare = stA, stB, stC
            wo_pre = []
            for t in range(TIME_STEPS):
                if t == TIME_STEPS - 1:
                    for oc in range(3):
                        wo = blkp.tile([P, KC, P], f16, tag="wo", name=f"wo{oc}")
                        nc.sync.dma_start(wo, w_out_blk[oc])
                        wo_pre.append(wo)
                if t > 0:
                    # s_t = tanh(state_t) into the dead buffer (old state_{t-1})
                    for k in range(KC):
                        nc.scalar.activation(curs[k], cur[k], Act.Tanh)
                def emit_B(ncb, jb, nz, psA):
                    psB = pspool.tile([P, TPC], f32, tag="ps", name=f"psB{t}_{ncb}")
                    for k in range(KC):
                        nc.tensor.matmul(psB, jb[:, k * P:(k + 1) * P], curs[k],
                                         start=(k == 0), stop=(k == KC - 1))
                    # sn = noise*T01 + signal ; d = lam*(s@Jm)*s ; d += sn
                    sn = epool.tile([P, TPC], f32, tag="epi", name=f"sn{t}_{ncb}")
                    nc.vector.scalar_tensor_tensor(
                        sn, nz, t01[:, ncb:ncb + 1], psA, Alu.mult, Alu.add)
                    d = epool.tile([P, TPC], f32, tag="epi", name=f"d{t}_{ncb}")
                    nc.vector.scalar_tensor_tensor(
                        d, psB, lam_sb[:, 0:1], curs[ncb], Alu.mult, Alu.mult)
                    nc.vector.tensor_tensor(d, d, sn, Alu.add)
                    nc.scalar.activation(spare[ncb], d, Act.Tanh)

                # B-groups are emitted one group behind A-groups so the PE
                # (in-order) has 2 A-groups of work while ACT produces the
                # step's s = tanh(state) chunks and the first J block streams
                pend = None
                for ncb in range(KC):
                    jb = wstp.tile([P, N], f16, tag="jset", name=f"jb{t}_{ncb}")
                    nc.sync.dma_start(
                        jb, j_blk[ncb].rearrange("p k c -> p (k c)"))
                    mb = wstp.tile([P, N], f16, tag="mset", name=f"mb{t}_{ncb}")
                    nc.sync.dma_start(
                        mb, m_blk[ncb].rearrange("p k c -> p (k c)"))
                    if t == 0:
                        wb = wsetp.tile([P, N], f16, tag="wset", name=f"wb{ncb}")
                        nc.sync.dma_start(
                            wb, w_blkd[ncb].rearrange("p k c -> p (k c)"))
                        ew = wpool.tile([P, N], f16, tag=f"effw{ncb}",
                                        name=f"effw{ncb}")
                        nc.vector.tensor_tensor(ew, wb, mb, Alu.mult)
                        effw_blk[ncb] = ew
                    nc.vector.tensor_tensor(jb, jb, mb, Alu.mult)
                    nz = npool.tile([P, TPC], f16, tag="nz", name=f"nz{t}_{ncb}")
                    nc.sync.dma_start(nz, noiseT[t, ncb * P:(ncb + 1) * P, :])
                    psA = pspool.tile([P, TPC], f32, tag="ps", name=f"psA{t}_{ncb}")
                    ewt = effw_blk[ncb]
                    for k in range(KC):
                        nc.tensor.matmul(psA, ewt[:, k * P:(k + 1) * P],
                                         cur[k], start=(k == 0), stop=(k == KC - 1))
                    if pend is not None:
                        emit_B(*pend)
                    pend = (ncb, jb, nz, psA)
                emit_B(*pend)
                cur, curs, spare = spare, cur, curs

            # ---- output projection: y = state @ W_out.T + b_out ----
            for oc in range(KO):
                wo = wo_pre[oc] if oc < len(wo_pre) else None
                if wo is None:
                    wo = blkp.tile([P, KC, P], f16, tag="wo")
                    nc.sync.dma_start(wo, w_out_blk[oc])
                ps = pspool.tile([P, TPC], f32, tag="ps")
                for k in range(KC):
                    nc.tensor.matmul(ps, wo[:, k, :], cur[k],
                                     start=(k == 0), stop=(k == KC - 1))
                yt = ypool.tile([P, TPC], f32, tag="y")
                nc.scalar.activation(yt, ps, Act.Identity,
                                     bias=bout_sb[:, oc:oc + 1])
                nc.sync.dma_start(yT[oc * P:(oc + 1) * P, :], yt)

    nc.compile()
    return nc


def _get_program():
    global _PROG
    if _PROG is None:
        _PROG = _build_program()
    return _PROG


def kernel(**inputs):
    from concourse.bass_utils import run_bass_kernel_spmd

    x = np.ascontiguousarray(np.asarray(inputs["x"], dtype=np.float32))
    W_in = np.asarray(inputs["W_in"], dtype=np.float32)
    b_in = np.asarray(inputs["b_in"], dtype=np.float32)
    weights = np.asarray(inputs["weights"], dtype=np.float32)
    J = np.asarray(inputs["J"], dtype=np.float32)
    theta = np.asarray(inputs["theta"], dtype=np.float32)
    lam = np.float32(np.asarray(inputs["lam"], dtype=np.float32))
    mask = np.asarray(inputs["mask"], dtype=np.float32)
    noise_raw = np.asarray(inputs["noise_raw"], dtype=np.float32)
    W_out = np.asarray(inputs["W_out"], dtype=np.float32)
    b_out = np.asarray(inputs["b_out"], dtype=np.float32)
    assert int(np.asarray(inputs["time_steps"])) == TIME_STEPS
    assert x.shape == (TOKENS, IN_DIM)

    f16 = np.float16

    def c(a):
        return np.ascontiguousarray(a)

    # replicated tensors (layout/dtype prep only; all arithmetic on device)
    w_in_blk = c(W_in.reshape(KC, P, KI, P).transpose(0, 3, 2, 1).astype(f16))
    w_out_blk = c(W_out.reshape(KO, P, KC, P).transpose(0, 3, 2, 1).astype(f16))
    w_blkd = c(weights.reshape(KC, P, KC, P).transpose(2, 1, 0, 3).astype(f16))
    j_blk = c(J.reshape(KC, P, KC, P).transpose(2, 1, 0, 3).astype(f16))
    m_blk = c(mask.reshape(KC, P, KC, P).transpose(2, 1, 0, 3).astype(f16))
    consts_t = c(np.concatenate([
        b_in.reshape(KC, P).T, b_out.reshape(KO, P).T,
        theta.reshape(KC, P).T,
        np.broadcast_to(lam, (P, 1)),
    ], axis=1).astype(np.float32))

    shared = {
        "w_in_blk": w_in_blk, "w_out_blk": w_out_blk,
        "w_blkd": w_blkd,
        "j_blk": j_blk, "m_blk": m_blk,
        "consts_t": consts_t,
    }

    in_maps = []
    for core in range(N_CORES):
        sl = slice(core * TPC, (core + 1) * TPC)
        in_maps.append({
            **shared,
            "xT": c(x[sl].T.astype(f16)),
            "noiseT": c(noise_raw[:, sl, :].transpose(0, 2, 1).astype(f16)),
        })

    nc = _get_program()
    res = run_bass_kernel_spmd(nc, in_maps, core_ids=list(range(N_CORES)))
    out = np.empty((TOKENS, OUT_DIM), dtype=np.float32)
    for core in range(N_CORES):
        out[core * TPC:(core + 1) * TPC] = res.results[core]["yT"].T
    return out
